# revision 17
# baseline (speedup 1.0000x reference)
"""Multi-head attention with Llama RoPE on 8 TRN2 NeuronCores.

Problem: x [2, 2048, 1024] f32; w_qkv [3072, 1024]; w_out [1024, 1024];
b_out [1024].  16 heads x head_dim 64, full (non-causal) softmax attention.

Sharding: 8 cores = 2 batches x 4 head-groups (4 heads per core).
Each core computes q/k/v projections for its 4 heads, RoPE, attention,
and a partial output projection over its 256 local features.  The host
sums the 4 partials per batch (row-parallel out-projection); the bias is
injected on one core per batch group.

Core kernel layout choices (see matmul as out[M,N] = lhsT[K,M].T @ rhs[K,N]):
 - x is fed transposed (xT [1024 d, 2048 s]); q/k are produced transposed
   (qT/kT [e, s]) directly by putting the weights stationary.
 - RoPE pair-swap (x1,x2 interleaved) is done with a 128x128 permutation
   matmul (swap adjacent partitions), then rope = q*C + swap(q)*S with
   per-partition cos/sin tables from the host.  The rope'd head dim is the
   reference's layout permuted identically for q and k, which leaves
   scores unchanged.
 - Scores are computed transposed (scoresT [kj, qi]) so the softmax axis
   is the partition axis; softmax denominators come from an extra ones
   column appended to v (PV matmul row 64), normalization is deferred to
   a rank-1 broadcast multiply after PV.
 - exp has no max-subtraction: scores are ~N(0,1) (max ~7), safe in f32.
 - All matmuls run as float32r (full-rate fp32 streaming, N>=256).
"""
import sys

sys.path.insert(0, "/opt/trn_rl_repo")

from contextlib import ExitStack

import numpy as np

import concourse.bass as bass
import concourse.tile as tile
from concourse import bacc, mybir
from concourse.bass2jax import (_bass_exec_p, install_neuronx_cc_hook,
                                partition_id_tensor)

F32 = mybir.dt.float32
F32R = mybir.dt.float32r

B, S, D = 2, 2048, 1024
H, HD = 16, 64          # global heads, head dim
HL = 4                  # heads per core
EL = HL * HD            # 256 local e-dims for q, k, v each
N_CORES = 8
SC = 512                # moving-dim chunk (s and e chunks)
N_SC = S // SC          # 4
N_ST = S // 128         # 16 s-tiles
N_DT = D // 128         # 8 d-chunks
N_KT = S // 128         # 16 kj-tiles


def r(ap):
    return ap.bitcast(F32R)


def build_kernel(repeat=1):
    nc = bacc.Bacc(None, target_bir_lowering=False)

    xT_ext = nc.declare_dram_parameter("xT", [D, S], F32R, isOutput=False)
    wqk_ext = nc.declare_dram_parameter("wqk", [D, 2 * EL], F32R, isOutput=False)
    wv_ext = nc.declare_dram_parameter("wv", [D, EL], F32R, isOutput=False)
    cos2_ext = nc.declare_dram_parameter("cos2", [128, S], F32, isOutput=False)
    sin2_ext = nc.declare_dram_parameter("sin2", [128, S], F32, isOutput=False)
    psw_ext = nc.declare_dram_parameter("psw", [128, 128], F32R, isOutput=False)
    wo_ext = nc.declare_dram_parameter("wo", [EL, D], F32R, isOutput=False)
    bias_ext = nc.declare_dram_parameter("bias", [128, D], F32, isOutput=False)
    onesv_ext = nc.declare_dram_parameter("onesv", [128, 4 * 65], F32R,
                                          isOutput=False)
    ones64_ext = nc.declare_dram_parameter("ones64p", [1, 64], F32R,
                                           isOutput=False)
    out_ext = nc.declare_dram_parameter("out", [S, D], F32, isOutput=True)

    with tile.TileContext(nc) as tc, ExitStack() as ctx, \
            nc.allow_low_precision(reason="float32r rounding writes"):
        # ---- persistent SBUF ----
        singles = ctx.enter_context(tc.tile_pool(name="singles", bufs=1))
        xT = [singles.tile([128, S], F32R, name=f"xT{i}") for i in range(N_DT)]
        wqk = [singles.tile([128, 2 * EL], F32R, name=f"wqk{i}") for i in range(N_DT)]
        wv = [singles.tile([128, EL], F32R, name=f"wv{i}") for i in range(N_DT)]
        cos2 = singles.tile([128, S], F32, name="cos2")
        sin2 = singles.tile([128, S], F32, name="sin2")
        psw = singles.tile([128, 128], F32R, name="psw")
        wo = [singles.tile([128, D], F32R, name=f"wo{i}") for i in range(2)]
        bias = singles.tile([128, D], F32, name="bias")
        ones64 = singles.tile([1, 64], F32R, name="ones64")
        # q/k transposed [e_local, s]; 2 tiles each of 2 heads
        qT = [singles.tile([128, S], F32, name=f"qT{i}") for i in range(2)]
        kT = [singles.tile([128, S], F32, name=f"kT{i}") for i in range(2)]
        # v natural [s, 4*(64+1)] with ones column per head
        vsb = [singles.tile([128, 4 * 65], F32R, name=f"v{i}") for i in range(N_ST)]
        # normalized attention output, transposed [d_local, s]
        onrm = [singles.tile([128, S], F32, name=f"onrm{i}") for i in range(2)]

        for i in range(N_DT):
            nc.sync.dma_start(out=xT[i][:], in_=xT_ext[128 * i:128 * (i + 1), :])
            nc.sync.dma_start(out=wqk[i][:], in_=wqk_ext[128 * i:128 * (i + 1), :])
            nc.sync.dma_start(out=wv[i][:], in_=wv_ext[128 * i:128 * (i + 1), :])
        nc.sync.dma_start(out=cos2[:], in_=cos2_ext[:])
        nc.sync.dma_start(out=sin2[:], in_=sin2_ext[:])
        nc.sync.dma_start(out=psw[:], in_=psw_ext[:])
        for i in range(2):
            nc.sync.dma_start(out=wo[i][:], in_=wo_ext[128 * i:128 * (i + 1), :])
        nc.sync.dma_start(out=bias[:], in_=bias_ext[:])
        nc.sync.dma_start(out=ones64[:], in_=ones64_ext[:])
        for st in range(N_ST):
            nc.sync.dma_start(out=vsb[st][:], in_=onesv_ext[:])

        # ---- pools ----
        pp = ctx.enter_context(tc.tile_pool(name="pp", bufs=3, space="PSUM"))
        pv_ps = ctx.enter_context(tc.tile_pool(name="pv_ps", bufs=4, space="PSUM"))
        sb3 = ctx.enter_context(tc.tile_pool(name="sb3", bufs=2))
        att_sb = ctx.enter_context(tc.tile_pool(name="att_sb", bufs=6))
        outev = ctx.enter_context(tc.tile_pool(name="outev", bufs=2))
        sb2 = ctx.enter_context(tc.tile_pool(name="sb2", bufs=3))

        for _rep in range(repeat):
            emit_body(nc, tc, ctx, locals())
    nc.finalize()
    return nc


def emit_body(nc, tc, ctx, env):
    (xT, wqk, wv, cos2, sin2, psw, wo, bias, ones64, qT, kT, vsb, onrm,
     out_ext, pp, pv_ps, sb3, att_sb, outev, sb2) = (
        env['xT'], env['wqk'], env['wv'], env['cos2'], env['sin2'], env['psw'],
        env['wo'], env['bias'], env['ones64'], env['qT'], env['kT'],
        env['vsb'], env['onrm'], env['out_ext'], env['pp'], env['pv_ps'],
        env['sb3'], env['att_sb'], env['outev'], env['sb2'])
    if True:
        # ---- phase 1b: v projection (natural layout + ones cols) ----
        for st in range(N_ST):
            ps = pp.tile([128, EL], F32, name="vproj", tag="ps")
            for dt_ in range(N_DT):
                nc.tensor.matmul(
                    ps[:],
                    xT[dt_][:, 128 * st:128 * (st + 1)],
                    wv[dt_][:],
                    start=(dt_ == 0), stop=(dt_ == N_DT - 1),
                )
            dst = vsb[st][:].rearrange("p (h e) -> p h e", h=4)[:, :, 0:64]
            nc.vector.tensor_copy(out=dst, in_=ps[:].rearrange("p (h e) -> p h e", h=4))

        inv_sqrt_hd = 1.0 / np.sqrt(HD)
        # per tile-pair: project q/k, rope them, then run their 2 heads
        for tq_ in range(2):
            for t in (tq_, tq_ + 2):           # q tile, then matching k tile
                dst = qT[t] if t < 2 else kT[t - 2]
                for cg in range(N_SC // 2):
                    pss = [pp.tile([128, SC], F32, name="proj", tag="ps")
                           for _ in range(2)]
                    # d-chunk outer: the stationary wqk block is reused for
                    # both s-chunks before the next LDWEIGHTS
                    for dt_ in range(N_DT):
                        for ci in range(2):
                            c = 2 * cg + ci
                            nc.tensor.matmul(
                                pss[ci][:],
                                wqk[dt_][:, 128 * t:128 * (t + 1)],
                                xT[dt_][:, SC * c:SC * (c + 1)],
                                start=(dt_ == 0), stop=(dt_ == N_DT - 1),
                            )
                    for ci in range(2):
                        c = 2 * cg + ci
                        nc.scalar.copy(out=r(dst[:, SC * c:SC * (c + 1)]),
                                       in_=pss[ci][:])
                buf = dst
                for c in range(N_SC):
                    sl = slice(SC * c, SC * (c + 1))
                    sw = pp.tile([128, SC], F32, name="swap", tag="ps")
                    nc.tensor.matmul(sw[:], psw[:], r(buf[:, sl]),
                                     start=True, stop=True)
                    t1 = sb3.tile([128, SC], F32, name="ropet1")
                    nc.vector.tensor_mul(t1[:], buf[:, sl], cos2[:, sl])
                    nc.vector.tensor_mul(r(buf[:, sl]), sw[:], sin2[:, sl])
                    nc.vector.tensor_add(r(buf[:, sl]), buf[:, sl], t1[:])
            for h in (2 * tq_, 2 * tq_ + 1):
                tq = h // 2
                ro = 64 * (h % 2)
                po = [None] * N_SC
                for kt in range(N_KT):
                    ksl = slice(128 * kt, 128 * (kt + 1))
                    ats = []
                    # all 4 QK matmuls share the same stationary k-tile,
                    # then all 4 PVs share the same stationary v-tile --
                    # grouping avoids an LDWEIGHTS per matmul
                    for c in range(N_SC):
                        qsl = slice(SC * c, SC * (c + 1))
                        ss = pp.tile([128, SC], F32, name="scoresT", tag="ps")
                        nc.tensor.matmul(
                            ss[:],
                            r(kT[tq][ro:ro + 64, ksl]),
                            r(qT[tq][ro:ro + 64, qsl]),
                            start=True, stop=True,
                        )
                        at = att_sb.tile([128, SC], F32, name="attnT")
                        nc.scalar.activation(out=r(at[:]), in_=ss[:],
                                             func=mybir.ActivationFunctionType.Exp,
                                             scale=inv_sqrt_hd)
                        ats.append(at)
                    for c in range(N_SC):
                        if kt == 0:
                            po[c] = pv_ps.tile([65, SC], F32, name="pvacc")
                        nc.tensor.matmul(
                            po[c][:],
                            vsb[kt][:, 65 * h:65 * h + 65],
                            r(ats[c][:]),
                            start=(kt == 0), stop=(kt == N_KT - 1),
                        )
                for c in range(N_SC):
                    rec = sb2.tile([1, SC], F32, name="recip", tag="nrm")
                    nc.vector.reciprocal(out=r(rec[:]), in_=po[c][64:65, :])
                    bc = pp.tile([64, SC], F32, name="bcast", tag="ps")
                    nc.tensor.matmul(bc[:], ones64[:], r(rec[:]),
                                     start=True, stop=True)
                    bs = sb2.tile([64, SC], F32, name="bcast_sb", tag="nrm")
                    nc.scalar.copy(out=bs[:], in_=bc[:])
                    nc.vector.tensor_mul(
                        r(onrm[tq][ro:ro + 64, SC * c:SC * (c + 1)]),
                        po[c][0:64, :], bs[:],
                    )

    # ---- phase 4: output projection (partial over local 256 d) ----
        for st in range(N_ST):
            ssl = slice(128 * st, 128 * (st + 1))
            pss = [pp.tile([128, SC], F32, name="oproj", tag="ps")
                   for _ in range(2)]
            for dt_ in range(2):
                for ec in range(2):
                    nc.tensor.matmul(
                        pss[ec][:],
                        r(onrm[dt_][:, ssl]),
                        wo[dt_][:, SC * ec:SC * (ec + 1)],
                        start=(dt_ == 0), stop=(dt_ == 1),
                    )
            for ec in range(2):
                esl = slice(SC * ec, SC * (ec + 1))
                ob = outev.tile([128, SC], F32, name="outev")
                nc.vector.tensor_add(ob[:], pss[ec][:], bias[:, esl])
                nc.sync.dma_start(out=out_ext[ssl, esl], in_=ob[:])




def run_spmd_per_device(nc, in_maps):
    """8 independent single-device executions of the same NEFF (the kernel
    is pure SPMD, no collectives; the axon terminal here hangs on
    multi-device shard_map, so we dispatch per-device jits asynchronously
    instead)."""
    import jax
    install_neuronx_cc_hook()
    devs = jax.devices()[:len(in_maps)]
    partition_name = (nc.partition_id_tensor.name
                      if nc.partition_id_tensor else None)
    in_names, out_names, out_avals, zero_outs = [], [], [], []
    for alloc in nc.m.functions[0].allocations:
        if not isinstance(alloc, mybir.MemoryLocationSet):
            continue
        name = alloc.memorylocations[0].name
        if alloc.kind == "ExternalInput":
            if name != partition_name:
                in_names.append(name)
        elif alloc.kind == "ExternalOutput":
            shape = tuple(alloc.tensor_shape)
            dtype = mybir.dt.np(alloc.dtype)
            out_names.append(name)
            out_avals.append(jax.core.ShapedArray(shape, dtype))
            zero_outs.append(np.zeros(shape, dtype))
    n_params = len(in_names)
    all_names = in_names + out_names
    if partition_name is not None:
        all_names = all_names + [partition_name]

    def _body(*args):
        operands = list(args)
        if partition_name is not None:
            operands.append(partition_id_tensor())
        outs = _bass_exec_p.bind(
            *operands,
            out_avals=tuple(out_avals),
            in_names=tuple(all_names),
            out_names=tuple(out_names),
            lowering_input_output_aliases=(),
            sim_require_finite=True,
            sim_require_nnan=True,
            nc=nc,
        )
        return tuple(outs)

    donate = tuple(range(n_params, n_params + len(out_names)))
    pending = []
    for i, in_map in enumerate(in_maps):
        f = jax.jit(_body, donate_argnums=donate, keep_unused=True,
                    device=devs[i])
        args = [np.asarray(in_map[k]) for k in in_names]
        args += [z.copy() for z in zero_outs]
        pending.append(f(*args))
    return [{name: np.asarray(outs[i]) for i, name in enumerate(out_names)}
            for outs in pending]


_ROPE_TABLES = None


def _tables():
    global _ROPE_TABLES
    if _ROPE_TABLES is None:
        inv_freq = 1.0 / (10000.0 ** (np.arange(0, HD, 2, dtype=np.float32) / HD))
        t = np.arange(S, dtype=np.float32)
        freqs = np.outer(t, inv_freq).astype(np.float32)  # [S, 32]
        cos, sin = np.cos(freqs), np.sin(freqs)
        # interleave pairs: row 2i and 2i+1 both get cos_i; sin row 2i = -s_i,
        # row 2i+1 = +s_i; tile 2 heads to fill 128 partitions
        c64 = np.repeat(cos.T, 2, axis=0)                 # [64, S]
        s64 = np.repeat(sin.T, 2, axis=0).copy()
        s64[0::2, :] *= -1.0
        cos2 = np.tile(c64, (2, 1)).astype(np.float32)    # [128, S]
        sin2 = np.tile(s64, (2, 1)).astype(np.float32)
        psw = np.zeros((128, 128), dtype=np.float32)
        idx = np.arange(128)
        psw[idx ^ 1, idx] = 1.0                           # out[j] = in[j^1]
        _ROPE_TABLES = (cos2, sin2, psw)
    return _ROPE_TABLES


_NC_CACHE = None
_last_in_maps = None


def kernel(x, w_qkv, w_out, b_out):
    global _NC_CACHE
    x = np.ascontiguousarray(x, dtype=np.float32)
    w_qkv = np.asarray(w_qkv, dtype=np.float32)
    w_out = np.asarray(w_out, dtype=np.float32)
    b_out = np.asarray(b_out, dtype=np.float32)

    cos2, sin2, psw = _tables()
    wq_g = w_qkv[0 * D:1 * D].reshape(H, HD, D)
    wk_g = w_qkv[1 * D:2 * D].reshape(H, HD, D)
    wv_g = w_qkv[2 * D:3 * D].reshape(H, HD, D)

    in_maps = []
    for c in range(N_CORES):
        b, g = divmod(c, 4)
        hs = slice(4 * g, 4 * g + 4)
        wq = wq_g[hs].reshape(EL, D)                       # [256, 1024]
        wk = wk_g[hs].reshape(EL, D)
        wv = wv_g[hs].reshape(EL, D)
        wqk = np.ascontiguousarray(
            np.concatenate([wq, wk], axis=0).T)            # [1024, 512]
        wvT = np.ascontiguousarray(wv.T)                   # [1024, 256]
        # w_out columns for local features, transposed -> [256 d_loc, 1024 e]
        wo = np.ascontiguousarray(w_out[:, 64 * 4 * g:64 * 4 * (g + 1)].T)
        bias = np.zeros((128, D), dtype=np.float32)
        if g == 0:
            bias[:] = b_out[None, :]
        onesv = np.zeros((128, 4 * 65), dtype=np.float32)
        onesv[:, 64::65] = 1.0
        in_maps.append({
            "onesv": onesv,
            "ones64p": np.ones((1, 64), dtype=np.float32),
            "xT": np.ascontiguousarray(x[b].T),
            "wqk": wqk,
            "wv": wvT,
            "cos2": cos2,
            "sin2": sin2,
            "psw": psw,
            "wo": wo,
            "bias": bias,
        })

    global _last_in_maps
    _last_in_maps = in_maps
    if _NC_CACHE is None:
        _NC_CACHE = build_kernel()
    res = run_spmd_per_device(_NC_CACHE, in_maps)
    outs = [res[c]["out"] for c in range(N_CORES)]
    full = np.empty((B, S, D), dtype=np.float32)
    full[0] = outs[0] + outs[1] + outs[2] + outs[3]
    full[1] = outs[4] + outs[5] + outs[6] + outs[7]
    return full
